# revision 1
# baseline (speedup 1.0000x reference)
"""Self-contained GCN encoder kernel for 8 TRN2 NeuronCores (Bass/Tile).

kernel(**inputs) takes the FULL unsharded inputs (as from setup_inputs())
and returns the FULL [50000, 64] float32 output.

Strategy: dst-node tiles of 128 are LPT-balanced across 8 cores (quantile-
matched slot order keeps the SPMD instruction stream identical); the NEFF is
specialized per run to the edge structure. Per core: embedding gathers +
feat^T assembly -> matmul -> dinv-scaled bf16 h1 table (emb_b handled via a
folded weight W1B = emb_b @ W1[64:128] and onehot matmuls); two split
AllGathers (slot ranges A/B, each table <= 32767 rows for int16 dma_gather
indices) exchange tables between cores; GCN aggregation gathers edge rows
with dma_gather, builds 0/1 selection matrices on-device (is_equal vs iota),
and segment-reduces on the TensorEngine into PSUM. The symmetric norm is
folded into the tables (src side) and the epilogue scale (dst side);
self-loops are a per-tile row add. B-table gather ops trail A-ops by 3
groups (phase-split PSUM accumulation with SBUF spill) so collectives
overlap gather work.
"""
import numpy as np
from concourse import bass, bacc, mybir, tile
from concourse.bass_utils import run_bass_kernel_spmd
from concourse.masks import make_identity

P = 128
CORES = 8
N = 50000
NTILES = 392
NPAD = NTILES * P     # 50176
TPC = NTILES // CORES  # 49
NLOC = TPC * P        # 6272
KSPLIT = 25           # slots 0..24 -> table A, 25..48 -> table B
NA = KSPLIT * P       # 3200 rows per core in A
NB = NLOC - NA        # 3072
TABA = NA * CORES     # 25600
TABB = NB * CORES     # 24576
C1 = 128
C2 = 64
IN_CH = 136
PAD_DSTL = 30000.0
G_MERGE = 4
DELAY_B = 3           # groups of A-ops run before each B-op


def wrap_idx(arr):
    return arr.reshape(-1, 16).T


def rup(x, m):
    return int((x + m - 1) // m * m)


def prep(x, edge_index, emb_a, emb_b, W1, b1, W2, b2):
    src, dst = np.asarray(edge_index[0]), np.asarray(edge_index[1])
    deg = np.bincount(dst, minlength=N).astype(np.float32) + 1.0
    dinv = (1.0 / np.sqrt(deg)).astype(np.float32)

    # ---- tile -> core assignment (LPT on edge counts) ----
    t_of_e = dst // P
    tile_cnt = np.bincount(t_of_e, minlength=NTILES)
    order = np.argsort(-tile_cnt, kind="stable")
    core_loads = np.zeros(CORES, dtype=np.int64)
    core_tiles = [[] for _ in range(CORES)]
    for t in order:
        c = int(np.argmin(core_loads))
        core_tiles[c].append(int(t))
        core_loads[c] += tile_cnt[t]
    c_of_t = np.zeros(NTILES, dtype=np.int64)
    k_of_t = np.zeros(NTILES, dtype=np.int64)
    for c in range(CORES):
        for k, t in enumerate(core_tiles[c]):
            c_of_t[t] = c
            k_of_t[t] = k

    # table coordinates: src node -> (which table, row)
    node_ids = np.arange(NPAD)
    nc_core = c_of_t[node_ids // P]
    nc_slot = k_of_t[node_ids // P]
    in_b = nc_slot >= KSPLIT
    trow = np.where(in_b,
                    nc_core * NB + (nc_slot - KSPLIT) * P + node_ids % P,
                    nc_core * NA + nc_slot * P + node_ids % P)

    # ---- sort edges by (core, slot, table) ----
    e_tab = in_b[src].astype(np.int64)
    e_row = trow[src]
    e_k = k_of_t[t_of_e]
    key = (c_of_t[t_of_e] * TPC + e_k) * 2 + e_tab
    sort = np.argsort(key, kind="stable")
    row_s = e_row[sort]
    dstl_s = (dst % P).astype(np.float32)[sort]
    bounds = np.searchsorted(key[sort], np.arange(CORES * TPC * 2 + 1))

    # ---- op schedule: per (group of G_MERGE slots, table): contiguous pack ----
    raw_ops = []   # (h, k0, k1, num_idxs, idxcol_off, [(j, k, paircol)...])
    idxcol_off = 0
    paircol = 0
    pairs_of_tile = np.zeros(TPC, dtype=np.int64)
    for k0 in range(0, TPC, G_MERGE):
        k1 = min(k0 + G_MERGE, TPC)
        for h in (0, 1):
            m_op = [sum(int(bounds[(c * TPC + k) * 2 + h + 1] -
                            bounds[(c * TPC + k) * 2 + h])
                        for k in range(k0, k1)) for c in range(CORES)]
            n = rup(max(m_op), P) // P
            if n == 0:
                continue
            pairset = set()
            for c in range(CORES):
                off = 0
                for k in range(k0, k1):
                    g = (c * TPC + k) * 2 + h
                    m = int(bounds[g + 1] - bounds[g])
                    if m == 0:
                        continue
                    for j in range(off // P, (off + m - 1) // P + 1):
                        pairset.add((j, k))
                    off += m
            pairlist = []
            for (j, k) in sorted(pairset):
                pairlist.append((j, k, paircol))
                pairs_of_tile[k] += 1
                paircol += 1
            raw_ops.append((h, k0, k1, n * P, idxcol_off, pairlist))
            idxcol_off += n * P // 16
    GCOLS = idxcol_off
    NPAIRS = paircol
    MAXCH = max(op[3] // P for op in raw_ops)
    n_rows_tot = sum(op[3] for op in raw_ops)
    assert all(pairs_of_tile > 0)

    # delayed-B emission order: A-ops stream, B-ops trail by DELAY_B groups
    a_ops = [op for op in raw_ops if op[0] == 0]
    b_ops = [op for op in raw_ops if op[0] == 1]
    ops_seq = []
    bi = 0
    for gi, aop in enumerate(a_ops):
        ops_seq.append(aop)
        if gi >= DELAY_B and bi < len(b_ops):
            ops_seq.append(b_ops[bi])
            bi += 1
    ops_seq.extend(b_ops[bi:])

    # ---- per-core arrays ----
    in_maps = []
    iota = np.tile(np.arange(P, dtype=np.float32)[None, :], (P, 1))
    for c in range(CORES):
        gidx16 = np.zeros((16, GCOLS), dtype=np.int16)
        dstlm = np.full((P, NPAIRS), PAD_DSTL, dtype=np.float32)
        for (h, k0, k1, num_idxs, coff, pairlist) in raw_ops:
            idx = np.zeros(num_idxs, dtype=np.int16)
            tilearr = np.full(num_idxs, -1, dtype=np.int64)
            dl = np.full(num_idxs, PAD_DSTL, dtype=np.float32)
            off = 0
            for k in range(k0, k1):
                g = (c * TPC + k) * 2 + h
                lo, hi = bounds[g], bounds[g + 1]
                m = int(hi - lo)
                if m == 0:
                    continue
                idx[off:off + m] = row_s[lo:hi].astype(np.int16)
                tilearr[off:off + m] = k
                dl[off:off + m] = dstl_s[lo:hi]
                off += m
            gidx16[:, coff:coff + num_idxs // 16] = wrap_idx(idx)
            for (j, k, pc_) in pairlist:
                seg_t = tilearr[j * P:(j + 1) * P]
                seg_d = dl[j * P:(j + 1) * P]
                dstlm[:, pc_] = np.where(seg_t == k, seg_d, PAD_DSTL)
        gidx = np.tile(gidx16, (8, 1))

        nodes = np.concatenate(
            [t * P + np.arange(P) for t in core_tiles[c]])
        valid = nodes < N
        nodes_c = np.where(valid, nodes, 0)
        x_own = np.where(valid[:, None], np.asarray(x)[nodes_c], 0.0).astype(np.float32)
        x_ownT = np.ascontiguousarray(x_own.T).astype(np.float32)
        dinv_own = np.where(valid, dinv[nodes_c], 1.0).astype(np.float32)
        dinvc = dinv_own.reshape(TPC, P).T.copy()

        xa = x_own[:, 0].astype(np.int64)
        eia = np.tile(wrap_idx(xa.astype(np.int16)), (8, 1))
        xbcol = x_own[:, 1].astype(np.float32).reshape(TPC, P).T.copy()

        import ml_dtypes
        in_maps.append({
            "x_ownT": x_ownT[2:10].astype(ml_dtypes.bfloat16),
            "eia": eia.copy(),
            "xbcol": xbcol,
            "emb_a": np.asarray(emb_a, dtype=np.float32),
            "emb_bt": np.asarray(emb_b).T.copy().astype(ml_dtypes.bfloat16),
            "W1b": np.asarray(W1).astype(ml_dtypes.bfloat16),
            "W2": np.asarray(W2, dtype=np.float32),
            "b1f": np.tile(np.asarray(b1, dtype=np.float32)[None, :], (P, 1)),
            "b2f": np.tile(np.asarray(b2, dtype=np.float32)[None, :], (P, 1)),
            "dinvc": dinvc,
            "iota": iota,
            "gidx": gidx,
            "dstlm": dstlm,
        })

    meta = {"raw_ops": raw_ops, "ops_seq": ops_seq, "GCOLS": GCOLS,
            "NPAIRS": NPAIRS, "MAXCH": MAXCH,
            "core_tiles": core_tiles, "n_rows_tot": n_rows_tot}
    return in_maps, meta


def build(meta):
    ops_seq = meta["ops_seq"]
    GCOLS = meta["GCOLS"]
    NPAIRS = meta["NPAIRS"]
    MAXCH = meta["MAXCH"]
    f32 = mybir.dt.float32
    bf16 = mybir.dt.bfloat16
    i16 = mybir.dt.int16
    GS = G_MERGE  # slots per pacc bank tile

    nc = bacc.Bacc("TRN2", target_bir_lowering=False, debug=False,
                   num_devices=CORES)
    x_ownT = nc.dram_tensor("x_ownT", [8, NLOC], bf16, kind="ExternalInput")
    eia = nc.dram_tensor("eia", [P, NLOC // 16], i16, kind="ExternalInput")
    xbcol = nc.dram_tensor("xbcol", [P, TPC], f32, kind="ExternalInput")
    emb_a = nc.dram_tensor("emb_a", [1000, 64], f32, kind="ExternalInput")
    emb_bt = nc.dram_tensor("emb_bt", [64, 50], bf16, kind="ExternalInput")
    W1b = nc.dram_tensor("W1b", [IN_CH, C1], bf16, kind="ExternalInput")
    W2 = nc.dram_tensor("W2", [C1, C2], f32, kind="ExternalInput")
    b1f = nc.dram_tensor("b1f", [P, C1], f32, kind="ExternalInput")
    b2f = nc.dram_tensor("b2f", [P, C2], f32, kind="ExternalInput")
    dinvc = nc.dram_tensor("dinvc", [P, TPC], f32, kind="ExternalInput")
    iota = nc.dram_tensor("iota", [P, P], f32, kind="ExternalInput")
    gidx = nc.dram_tensor("gidx", [P, GCOLS], i16, kind="ExternalInput")
    dstlm = nc.dram_tensor("dstlm", [P, NPAIRS], f32, kind="ExternalInput")
    y = nc.dram_tensor("y", [NLOC, C2], f32, kind="ExternalOutput")

    with tile.TileContext(nc) as tc:
        with tc.tile_pool(name="const", bufs=1) as cpool, \
             tc.tile_pool(name="meta", bufs=1) as mpool, \
             tc.tile_pool(name="emb", bufs=1) as epool, \
             tc.tile_pool(name="feat", bufs=3) as fpool, \
             tc.tile_pool(name="he1", bufs=3) as he1pool, \
             tc.tile_pool(name="he2", bufs=3) as he2pool, \
             tc.tile_pool(name="sel", bufs=4) as spool, \
             tc.tile_pool(name="epi", bufs=3) as tpool, \
             tc.tile_pool(name="part", bufs=30) as partp, \
             tc.tile_pool(name="ptr", bufs=2, space="PSUM") as ptrp, \
             tc.tile_pool(name="pmm", bufs=1, space="PSUM") as pmmp, \
             tc.tile_pool(name="pacc", bufs=5, space="PSUM") as paccp, \
             tc.tile_pool(name="dram", bufs=1, space="DRAM") as dram:

            # ---------- constants ----------
            ident = cpool.tile([P, P], f32, tag="ident")
            make_identity(nc, ident[:])
            identb = cpool.tile([P, P], bf16, tag="identb")
            nc.vector.tensor_copy(out=identb[:], in_=ident[:])
            iota_t = cpool.tile([P, P], f32, tag="iota")
            nc.sync.dma_start(out=iota_t[:], in_=iota[:])
            W1lo = cpool.tile([P, C1], bf16, tag="w1lo")
            nc.sync.dma_start(out=W1lo[:], in_=W1b[0:P, :])
            W1hi = cpool.tile([IN_CH - P, C1], bf16, tag="w1hi")
            nc.sync.dma_start(out=W1hi[:], in_=W1b[P:IN_CH, :])
            W2t = cpool.tile([C1, C2], f32, tag="w2")
            nc.sync.dma_start(out=W2t[:], in_=W2[:])
            b1t = cpool.tile([P, C1], f32, tag="b1")
            nc.sync.dma_start(out=b1t[:], in_=b1f[:])
            b2t = cpool.tile([P, C2], f32, tag="b2")
            nc.sync.dma_start(out=b2t[:], in_=b2f[:])
            dinv_t = cpool.tile([P, TPC], f32, tag="dinv")
            nc.sync.dma_start(out=dinv_t[:], in_=dinvc[:])
            xb_t = cpool.tile([P, TPC], f32, tag="xb")
            nc.sync.dma_start(out=xb_t[:], in_=xbcol[:])
            embBT = cpool.tile([64, 50], bf16, tag="embBT")
            nc.sync.dma_start(out=embBT[:], in_=emb_bt[:])
            eia_t = mpool.tile([P, NLOC // 16], i16, tag="eia")
            nc.sync.dma_start(out=eia_t[:], in_=eia[:])

            agA1 = dram.tile([NA, C1], bf16, tag="agA1")
            agB1 = dram.tile([NB, C1], bf16, tag="agB1")
            tabA1 = dram.tile([TABA, C1], bf16, tag="tabA1")
            tabB1 = dram.tile([TABB, C1], bf16, tag="tabB1")
            agA2 = dram.tile([NA, C2], f32, tag="agA2")
            agB2 = dram.tile([NB, C2], f32, tag="agB2")
            tabA2 = dram.tile([TABA, C2], f32, tag="tabA2")
            tabB2 = dram.tile([TABB, C2], f32, tag="tabB2")

            def slot_dst1(k):
                return (agA1, k * P) if k < KSPLIT else (agB1, (k - KSPLIT) * P)

            def slot_dst2(k):
                return (agA2, k * P) if k < KSPLIT else (agB2, (k - KSPLIT) * P)

            # ---------- stage 1 ----------
            ga = epool.tile([P, TPC * 64], f32, tag="ga")
            gab = epool.tile([P, TPC * 64], bf16, tag="gab")
            spe = 7
            for e in range((TPC + spe - 1) // spe):
                k0e, k1e = e * spe, min((e + 1) * spe, TPC)
                nn = (k1e - k0e) * P
                nc.gpsimd.dma_gather(
                    out_ap=ga[:, k0e * 64:k1e * 64].rearrange(
                        "p (n c) -> p n c", c=64),
                    in_ap=emb_a[:],
                    idxs_ap=eia_t[:, k0e * P // 16:k1e * P // 16],
                    num_idxs=nn, num_idxs_reg=nn, elem_size=64,
                    single_packet=(nn <= 1024))
                nc.vector.tensor_copy(out=gab[:, k0e * 64:k1e * 64],
                                      in_=ga[:, k0e * 64:k1e * 64])
            W1mid = cpool.tile([64, C1], bf16, tag="w1mid")
            nc.sync.dma_start(out=W1mid[:], in_=W1b[64:128, :])
            pWB = ptrp.tile([P, P], f32, space="PSUM", tag="ptr")
            nc.tensor.matmul(out=pWB[0:50, :], lhsT=embBT[:],
                             rhs=W1mid[:], start=True, stop=True)
            W1Bp = cpool.tile([50, C1], bf16, tag="w1bp")
            nc.vector.tensor_copy(out=W1Bp[:], in_=pWB[0:50, :])
            gidx_t = mpool.tile([P, GCOLS], i16, tag="gidx")
            nc.sync.dma_start(out=gidx_t[:], in_=gidx[:])
            dstl_t = mpool.tile([P, NPAIRS], f32, tag="dstl")
            nc.sync.dma_start(out=dstl_t[:], in_=dstlm[:])

            for k in range(TPC):
                ptrA = ptrp.tile([P, P], bf16, space="PSUM", tag="ptr")
                nc.tensor.transpose(out=ptrA[0:64, :], in_=gab[:, k * 64:(k + 1) * 64],
                                    identity=identb[:])
                gaT = fpool.tile([64, P], bf16, tag="gaT")
                nc.vector.tensor_copy(out=gaT[:], in_=ptrA[0:64, :])
                oneB = fpool.tile([P, 64], bf16, tag="oneB")
                nc.vector.tensor_tensor(
                    out=oneB[:, 0:50], in0=xb_t[:, k:k + 1].to_broadcast([P, 50]),
                    in1=iota_t[:, 0:50], op=mybir.AluOpType.is_equal)
                pB = ptrp.tile([P, P], bf16, space="PSUM", tag="ptr")
                nc.tensor.transpose(out=pB[0:50, :], in_=oneB[:, 0:50],
                                    identity=identb[:])
                oneBT = fpool.tile([50, P], bf16, tag="oneBT")
                nc.vector.tensor_copy(out=oneBT[:], in_=pB[0:50, :])
                fThi = fpool.tile([8, P], bf16, tag="fthi")
                nc.sync.dma_start(out=fThi[:], in_=x_ownT[:, k * P:(k + 1) * P])
                ph1 = pmmp.tile([P, C1], f32, space="PSUM", tag="pmm")
                nc.tensor.matmul(out=ph1[:], lhsT=gaT[:], rhs=W1lo[0:64, :],
                                 start=True, stop=False)
                nc.tensor.matmul(out=ph1[:], lhsT=oneBT[:], rhs=W1Bp[:],
                                 start=False, stop=False)
                nc.tensor.matmul(out=ph1[:], lhsT=fThi[:], rhs=W1hi[:],
                                 start=False, stop=True)
                h1s = tpool.tile([P, C1], bf16, tag="h1s")
                nc.scalar.activation(out=h1s[:], in_=ph1[:],
                                     func=mybir.ActivationFunctionType.Copy,
                                     scale=dinv_t[:, k:k + 1])
                dstt, off = slot_dst1(k)
                nc.sync.dma_start(out=dstt[off:off + P, :], in_=h1s[:])
                if k == KSPLIT - 1:
                    nc.gpsimd.collective_compute(
                        "AllGather", mybir.AluOpType.bypass,
                        replica_groups=[list(range(CORES))],
                        ins=[agA1.opt()], outs=[tabA1.opt()])
            nc.gpsimd.collective_compute(
                "AllGather", mybir.AluOpType.bypass,
                replica_groups=[list(range(CORES))],
                ins=[agB1.opt()], outs=[tabB1.opt()])

            # ---------- conv passes ----------
            def conv(tabA, tabB, TA, TB, slot_dst, C, hepool, hetag, hedt, Sdt,
                     btile, last, agg_next=None, fire_b=None):
                npairs_of = {0: {}, 1: {}}
                for op in ops_seq:
                    for (j, k, pc_) in op[5]:
                        d = npairs_of[op[0]]
                        d[k] = d.get(k, 0) + 1
                done_of = {0: {k: 0 for k in npairs_of[0]},
                           1: {k: 0 for k in npairs_of[1]}}
                bank_of = {}      # (k, phase) -> psum tile
                partial_of = {}   # k -> sbuf partial from phase A

                def epilogue(k, pacc_ap):
                    srct, soff = slot_dst(k)
                    self_sb = tpool.tile([P, C], hedt, tag=f"self{C}",
                                         name=f"self_{C}_{k}")
                    nc.sync.dma_start(out=self_sb[:], in_=srct[soff:soff + P, :])
                    t1 = tpool.tile([P, C], f32, tag=f"t1{C}", name=f"t1_{C}_{k}")
                    nc.vector.tensor_add(out=t1[:], in0=pacc_ap, in1=self_sb[:])
                    if k in partial_of:
                        t1b = tpool.tile([P, C], f32, tag=f"t1b{C}",
                                         name=f"t1b_{C}_{k}")
                        nc.vector.tensor_add(out=t1b[:], in0=t1[:],
                                             in1=partial_of.pop(k)[:])
                        t1 = t1b
                    t2 = tpool.tile([P, C], f32, tag=f"t2{C}", name=f"t2_{C}_{k}")
                    nc.scalar.activation(out=t2[:], in_=t1[:],
                                         func=mybir.ActivationFunctionType.Copy,
                                         scale=dinv_t[:, k:k + 1])
                    t3 = tpool.tile([P, C], f32, tag=f"t3{C}", name=f"t3_{C}_{k}")
                    nc.vector.tensor_add(out=t3[:], in0=t2[:], in1=btile[:])
                    hrelu = tpool.tile([P, C], f32, tag=f"hr{C}", name=f"hr_{C}_{k}")
                    nc.vector.tensor_scalar_max(out=hrelu[:], in0=t3[:],
                                                scalar1=0.0)
                    if not last:
                        ptr2 = ptrp.tile([P, P], f32, space="PSUM", tag="ptr",
                                         name=f"ptr2_{k}")
                        nc.tensor.transpose(out=ptr2[:], in_=hrelu[:],
                                            identity=ident[:])
                        hT = fpool.tile([P, P], f32, tag="hT", name=f"hT_{k}")
                        nc.vector.tensor_copy(out=hT[:], in_=ptr2[:])
                        ph2 = pmmp.tile([P, C2], f32, space="PSUM", tag="pmm",
                                        name=f"ph2_{k}")
                        nc.tensor.matmul(out=ph2[:], lhsT=hT[:], rhs=W2t[:],
                                         start=True, stop=True)
                        h2s = tpool.tile([P, C2], f32, tag="h2s", name=f"h2s_{k}")
                        nc.scalar.activation(
                            out=h2s[:], in_=ph2[:],
                            func=mybir.ActivationFunctionType.Copy,
                            scale=dinv_t[:, k:k + 1])
                        d2, o2 = slot_dst2(k)
                        nc.sync.dma_start(out=d2[o2:o2 + P, :], in_=h2s[:])
                        if agg_next is not None:
                            agg_next(k)
                    else:
                        nc.sync.dma_start(out=y[k * P:(k + 1) * P, :],
                                          in_=hrelu[:])

                for opi, (h, k0, k1, num_idxs, coff, pairlist) in enumerate(ops_seq):
                    he = hepool.tile([P, MAXCH * C], hedt, tag=hetag,
                                     name=f"he_{C}_{h}_{k0}")
                    tab = tabB if h else tabA
                    nch = num_idxs // P
                    nc.gpsimd.dma_gather(
                        out_ap=he[:, 0:nch * C].rearrange(
                            "p (n c) -> p n c", c=C),
                        in_ap=tab[:],
                        idxs_ap=gidx_t[:, coff:coff + num_idxs // 16],
                        num_idxs=num_idxs, num_idxs_reg=num_idxs, elem_size=C,
                        single_packet=(num_idxs <= 1024))
                    for (j, k, pc_) in pairlist:
                        ph = h
                        if (k, ph) not in bank_of:
                            bank_of[(k, ph)] = paccp.tile(
                                [P, C], f32, space="PSUM",
                                tag="pacc", name=f"pacc_{C}_{ph}_{k}")
                        pacc_ap = bank_of[(k, ph)][:]
                        S = spool.tile([P, P], Sdt, tag=f"S{C}",
                                       name=f"S_{C}_{pc_}")
                        nc.vector.tensor_tensor(
                            out=S[:],
                            in0=dstl_t[:, pc_:pc_ + 1].to_broadcast([P, P]),
                            in1=iota_t[:],
                            op=mybir.AluOpType.is_equal)
                        nc.tensor.matmul(out=pacc_ap, lhsT=S[:],
                                         rhs=he[:, j * C:(j + 1) * C],
                                         start=(done_of[ph][k] == 0),
                                         stop=(done_of[ph][k] == npairs_of[ph][k] - 1))
                        done_of[ph][k] += 1
                        if done_of[ph][k] == npairs_of[ph][k]:
                            bank_of.pop((k, ph))
                            if ph == 0 and npairs_of[1].get(k, 0) > 0:
                                part = partp.tile([P, C], f32, tag=f"part{C}",
                                                  name=f"part_{C}_{k}")
                                nc.vector.tensor_copy(out=part[:], in_=pacc_ap)
                                partial_of[k] = part
                            else:
                                epilogue(k, pacc_ap)

                for hh in (0, 1):
                    assert all(done_of[hh][k] == npairs_of[hh][k]
                               for k in npairs_of[hh])
                assert not partial_of

            # between-conv collectives, fired as soon as enough slots finished
            def agg_next(k):
                if k == KSPLIT - 1:
                    nc.gpsimd.collective_compute(
                        "AllGather", mybir.AluOpType.bypass,
                        replica_groups=[list(range(CORES))],
                        ins=[agA2.opt()], outs=[tabA2.opt()])
                if k == TPC - 1:
                    nc.gpsimd.collective_compute(
                        "AllGather", mybir.AluOpType.bypass,
                        replica_groups=[list(range(CORES))],
                        ins=[agB2.opt()], outs=[tabB2.opt()])

            conv(tabA1, tabB1, TABA, TABB, slot_dst1, C1, he1pool, "he1",
                 bf16, bf16, b1t, last=False, agg_next=agg_next)
            conv(tabA2, tabB2, TABA, TABB, slot_dst2, C2, he2pool, "he2",
                 f32, f32, b2t, last=True)

    nc.compile()
    return nc


_cache = {}


def kernel(x, edge_index, emb_a, emb_b, W1, b1, W2, b2):
    in_maps, meta = prep(x, edge_index, emb_a, emb_b, W1, b1, W2, b2)
    key = (meta["GCOLS"], meta["NPAIRS"],
           tuple((op[0], op[1], op[2], op[3], op[4], tuple(op[5]))
                 for op in meta["ops_seq"]))
    if key not in _cache:
        _cache[key] = build(meta)
    nc = _cache[key]
    res = run_bass_kernel_spmd(nc, in_maps, core_ids=list(range(CORES)))
    out = np.zeros((N, C2), dtype=np.float32)
    for c in range(CORES):
        yc = res.results[c]["y"]
        nodes = np.concatenate(
            [t * P + np.arange(P) for t in meta["core_tiles"][c]])
        valid = nodes < N
        out[nodes[valid]] = yc[valid]
    return out

